# revision 11
# baseline (speedup 1.0000x reference)
"""Gumbel-softmax sample + symmetric scatter kernel for 8 trn2 NeuronCores.

Math: out[e] = sigmoid(((g0 - g1) + (gum0 - gum1)) / TEMP) with
gum_k = -log(-log(u_k + EPS) + EPS).  The scatter target is fully
deterministic: part 1 (first S*DEL_NUM elements) is a dense [S, DEL_NUM]
block at matrix[0:S, S:SZ]; part 2 is the strict upper triangle of the
bottom-right [DEL_NUM, DEL_NUM] block.  Output = matrix + matrix.T.

Device: each core computes a contiguous 1/8 of the E sigmoid values
(memory-bound elementwise map).  Host places the values into the
symmetric [SZ, SZ] output.
"""

import numpy as np

SZ = 8192
DEL_NUM = 2048
S = SZ - DEL_NUM               # 6144
E1 = S * DEL_NUM               # 12,582,912 dense block elements
E2 = DEL_NUM * (DEL_NUM - 1) // 2  # 2,096,128 triangular elements
E = E1 + E2                    # 14,679,040
NCORES = 8
CH = E // NCORES               # 1,834,880 elements per core
P = 128
FTOT = CH // P                 # 14,335 outputs per partition
NCHUNK = 5
F = FTOT // NCHUNK             # 2,867
TEMP = 10.0
EPS = 1e-20

_cache = {}

# Sigmoid placement: "batch" = all sigmoids after all Ln passes (2 ACT
# table loads total); "inline" = sigmoid right after each chunk's combine
# (2 loads per chunk, but output DMAs overlap input stream).
# "bf16" = batch schedule + bfloat16 output tile/DMA (halves output HBM
# traffic; host upcasts to f32; ~2e-3 Fro rel err, gate is 2e-2).
# "bf16c4" = bf16 + 4 chunks (F=3583): fewer, larger DMAs measured ~3 us
# per exec faster than the 5-chunk default on HW (97.7 vs 101 us).
VARIANT = "bf16c4"


def _build(variant=None, reps=1):
    """Build the bass program.  ``reps`` unrolls the whole kernel body
    that many times inside one NEFF (identical full work per rep, same
    in/out tensors) — used by test.py to time steady-state per-execution
    cost with the per-dispatch tunnel overhead amortized by reps."""
    import concourse.bacc as bacc
    import concourse.mybir as mybir
    import concourse.tile as tile

    f32 = mybir.dt.float32
    bf16 = mybir.dt.bfloat16
    AF = mybir.ActivationFunctionType

    nc = bacc.Bacc(
        "TRN2", target_bir_lowering=False, debug=False, num_devices=NCORES
    )

    # Float activation biases require registered const APs.
    for val in (EPS,):
        t = nc.alloc_sbuf_tensor(f"const-f32-{val}", [128, 1], f32)
        nc.gpsimd.memset(t.ap(), val)
        nc.const_aps.aps[(f32, val)] = t.ap()
    nc.all_engine_barrier()

    if variant is None:
        variant = VARIANT
    # Variant grammar: base "batch"/"inline"/"tail"/"split"/"ring2"/"buf3"
    # (f32 out), or "bf16[r2][cN]": bf16 out, optional out-DMA on the
    # second HWDGE ring (r2 -> nc.scalar), optional chunk count cN.
    out_dt = bf16 if variant.startswith("bf16") else f32
    nchunk = NCHUNK
    if variant.startswith("bf16") and "c" in variant and "tail" not in variant:
        nchunk = int(variant.split("c")[1])

    gen_ap = nc.dram_tensor("gen", [P, 2 * FTOT], f32, kind="ExternalInput").ap()
    u_ap = nc.dram_tensor("u", [P, 2 * FTOT], f32, kind="ExternalInput").ap()
    out_ap = nc.dram_tensor("out", [P, FTOT], out_dt, kind="ExternalOutput").ap()

    # "tail" variant: big chunks first, tiny last chunk -> the serial
    # chain behind the last-arriving input DMA (Ln,Ln,DVE*3,table
    # switch,Sigmoid,out-DMA) shrinks from ~25us to ~6us.  Big chunks'
    # sigmoids run before the last chunk's Lns so the final sigmoid
    # table load hides under the last chunk's DVE work.
    if variant == "tail" or (variant.startswith("bf16") and "tail" in variant):
        sizes = [3456, 3456, 3456, 3456, 511]
    else:
        base = FTOT // nchunk
        sizes = [base] * nchunk
        sizes[-1] += FTOT - base * nchunk
    offs = [sum(sizes[:i]) for i in range(len(sizes))]
    with tile.TileContext(nc) as tc:
        with tc.tile_pool(name="pool", bufs=2) as pool:
            for _r in range(reps):
                s_tiles = []
                for i in range(len(sizes)):
                    Fi, Oi = sizes[i], offs[i]
                    fs = slice(2 * Oi, 2 * (Oi + Fi))
                    ut = pool.tile([P, 2 * Fi], f32, tag="u", bufs=3 if variant == "buf3" else 2)
                    nc.sync.dma_start(ut[:], u_ap[:, fs])
                    gt = pool.tile([P, 2 * Fi], f32, tag="g", bufs=2)
                    # "d2": gen input stream rides the second HWDGE ring
                    # (ACT engine / qActDynamicHW) so the two input streams'
                    # descriptor generation runs in parallel.
                    in2 = nc.scalar if "d2" in variant else nc.sync
                    in2.dma_start(gt[:], gen_ap[:, fs])

                    # l1 = log(u + eps); w = max(-l1, 2^-24) (guards LUT error at
                    # u ~= 1 from sending a <=0 value into the second log);
                    # l2 = log(w) = -gumbel.  All in place in ut.
                    nc.scalar.activation(ut[:], ut[:], AF.Ln, bias=EPS)
                    nc.vector.tensor_scalar(
                        ut[:], ut[:], -1.0, 5.9604645e-08,
                        op0=mybir.AluOpType.mult, op1=mybir.AluOpType.max,
                    )
                    nc.scalar.activation(ut[:], ut[:], AF.Ln, bias=0.0)

                    lv = ut.rearrange("p (f two) -> p f two", two=2)
                    gv = gt.rearrange("p (f two) -> p f two", two=2)
                    s = pool.tile([P, Fi], f32, tag="s", bufs=len(sizes))
                    # s = (g0 - g1) + l2_1 - l2_0  (gum0 - gum1 = l2_1 - l2_0)
                    nc.vector.tensor_sub(s[:], gv[:, :, 0], gv[:, :, 1])
                    nc.vector.tensor_add(s[:], s[:], lv[:, :, 1])
                    nc.vector.tensor_sub(s[:], s[:], lv[:, :, 0])
                    if variant == "inline":
                        nc.scalar.activation(s[:], s[:], AF.Sigmoid, scale=1.0 / TEMP)
                        nc.sync.dma_start(out_ap[:, Oi : Oi + Fi], s[:])
                    else:
                        s_tiles.append((s, Oi, Fi))

                    # "split"/"tail": drain ready sigmoids before the last chunk
                    # so the final table switch is off the critical tail.
                    if variant in ("split", "tail") and i == len(sizes) - 2:
                        for sj, Oj, Fj in s_tiles:
                            nc.scalar.activation(sj[:], sj[:], AF.Sigmoid, scale=1.0 / TEMP)
                            nc.sync.dma_start(out_ap[:, Oj : Oj + Fj], sj[:])
                        s_tiles = []

                # Sigmoids batched last: one Ln->Sigmoid ACT table switch total.
                # "ring2": output DMAs ride the second HWDGE ring (qActDynamicHW)
                # so they interleave with the input stream at SDMA level.
                out_eng = nc.scalar if (variant == "ring2" or "r2" in variant) else nc.sync
                for s, Oi, Fi in s_tiles:
                    if out_dt is bf16:
                        sb = pool.tile([P, Fi], bf16, tag="sb", bufs=len(sizes))
                        nc.scalar.activation(sb[:], s[:], AF.Sigmoid, scale=1.0 / TEMP)
                        out_eng.dma_start(out_ap[:, Oi : Oi + Fi], sb[:])
                    else:
                        nc.scalar.activation(s[:], s[:], AF.Sigmoid, scale=1.0 / TEMP)
                        out_eng.dma_start(out_ap[:, Oi : Oi + Fi], s[:])

    nc.compile()
    return nc


def get_nc(variant=None, reps=1):
    key = (variant or VARIANT, reps)
    if key not in _cache:
        _cache[key] = _build(key[0], reps=key[1])
    return _cache[key]


def run_cores(gen: np.ndarray, u: np.ndarray, trace: bool = False):
    """Run the SPMD kernel on flat [E, 2] inputs; returns (flat out [E], results obj)."""
    from concourse.bass_utils import run_bass_kernel_spmd

    nc = get_nc()
    in_maps = []
    for c in range(NCORES):
        sl = slice(c * CH, (c + 1) * CH)
        in_maps.append(
            {
                "gen": gen[sl].reshape(P, 2 * FTOT),
                "u": u[sl].reshape(P, 2 * FTOT),
            }
        )
    kw = {}
    if trace:
        kw = {"trace": True, "trace_cores": list(range(NCORES)), "stitch_traces": True}
    res = run_bass_kernel_spmd(nc, in_maps, core_ids=list(range(NCORES)), **kw)
    out = np.concatenate(
        [np.asarray(r["out"]).astype(np.float32).reshape(-1) for r in res.results]
    )
    return out, res


def assemble(out: np.ndarray) -> np.ndarray:
    full = np.zeros((SZ, SZ), np.float32)
    a = out[:E1].reshape(S, DEL_NUM)
    full[:S, S:] = a
    full[S:, :S] = a.T
    ti, tj = np.triu_indices(DEL_NUM, k=1)
    b = np.zeros((DEL_NUM, DEL_NUM), np.float32)
    b[ti, tj] = out[E1:]
    full[S:, S:] = b + b.T
    return full


def kernel(gen_matrix=None, u=None, sz=None, del_num=None, **_ignored):
    gen = np.ascontiguousarray(np.asarray(gen_matrix, dtype=np.float32))
    uu = np.ascontiguousarray(np.asarray(u, dtype=np.float32))
    assert gen.shape == (E, 2) and uu.shape == (E, 2)
    out, _ = run_cores(gen, uu)
    return assemble(out)

